# revision 1
# baseline (speedup 1.0000x reference)
"""CapsuleLayer (dynamic routing, 3 iterations) Trainium2 Bass kernel.

Problem: inputs [64, 2048, 16] f32, W [1, 2048, 32, 16, 16] f32
  inputs_hat[b,n,o,d] = sum_i W[n,o,d,i] * inputs[b,n,i]
  3 routing iterations (softmax over o); only the last s/squash matters, and the
  b-update never uses `outputs`, so the whole computation collapses to:
    ihsum[b,n,o] = sum_d ih[b,n,o,d]            (= x . Wsum)
    e1 = exp(ihsum/32); c1 = e1 / sum_o e1
    b2 = ihsum*(1/32 + c1); e2 = exp(b2); r2 = 1/sum_o e2   (c2 = e2*r2)
    s[b,o,d] = sum_n c2[b,n,o] * ih[b,n,o,d]
    out = squash(s)

Sharding: Ni (2048) split 8 ways (256 capsules per core). Routing is local per
(b, n); each core produces a partial s [64, 32, 16] which the host sums and
squashes (tiny: 32K elements).

Per-core device pipeline (all matmuls bf16, PSUM f32), software-pipelined
one block (16 units = 8 quads) ahead:
  pass 1: ihsum via per-capsule matmuls, x stationary (4 capsules col-tiled,
          batch split in halves of 32), batched 16 units per PSUM bank.
  routing (staged across the previous block's pass 2; real-HW constraint:
          gpsimd cannot read PSUM and has no fused STT):
          e1 = ACT exp(ps1/32); z1,r1,r1s=32/z1 on DVE;
          v32 = 32*c1 = e1*r1s (gpsimd TT); t2raw = (v32+1)*ihsum
          (DVE fused STT, PSUM src); e2 = ACT exp(t2raw/32); z2,r2 on DVE.
  pass 2: per unit: ih into PSUM [128, 512]; weighting routes balanced
          under the PE floor: B = DVE STT (psum*r2)*e2, A = ACT copy*r2 +
          DVE TT*e2 (2x mode), C = ACT copy*r2 + gpsimd TT*e2; then a PE
          contraction with a fixed 0/1 selector [128->32] accumulating the
          batch-half h's partial s into ps_s[32h:32h+32] (PSUM, 2 groups).
  epilogue: ACT copies ps_s halves to SBUF, DMA to HBM; host sums the 8
          per-core partials and applies squash.
"""

import os
import sys

import numpy as np
import ml_dtypes

sys.path.insert(0, "/opt/trn_rl_repo")
sys.path.insert(0, "/opt/pypackages")

import concourse.bass as bass
import concourse.mybir as mybir
import concourse.tile as tile
from concourse import bacc
from concourse.bass_utils import run_bass_kernel_spmd

BF16 = mybir.dt.bfloat16
F32 = mybir.dt.float32
AF = mybir.ActivationFunctionType
OP = mybir.AluOpType

B, NI, DI, NO, DO = 64, 2048, 16, 32, 16
NCORES = 8
NL = NI // NCORES            # 256 capsules per core
OD = NO * DO                 # 512
NQ = NL // 4                 # 64 quads (4 capsules each)
NUNITS = NQ * 2              # 128 units: (quad, batch-half)
UNITS_PER_BLOCK = 16         # routing block: 16 units -> psum [128, 512]
NBLOCKS = NUNITS // UNITS_PER_BLOCK   # 8
WCHUNK_Q = 8                 # quads per W dma chunk (32 capsules)
EPS = 1e-7
WARM = int(os.environ.get('K_WARM', '0'))
# per 16 units: route 1 = DVE scalar_tensor_tensor (fused), route 2 =
# ACT copy + GPSIMD multiply, route 3 = ACT copy + DVE multiply (2x mode)
ROUTE1 = frozenset(range(8))
ROUTE2 = frozenset({14})


def _build_program():
    nc = bacc.Bacc("TRN2", target_bir_lowering=False, debug=False)

    x_d = nc.dram_tensor("x", [64, NQ, 2, 128], BF16, kind="ExternalInput").ap()
    w_d = nc.dram_tensor("w", [NQ, 64, OD], BF16, kind="ExternalInput").ap()
    ws_d = nc.dram_tensor("ws", [64, NQ, NO], BF16, kind="ExternalInput").ap()
    es_d = nc.dram_tensor("esel", [128, 32], BF16, kind="ExternalInput").ap()
    s_d = nc.dram_tensor("s_out", [64, OD], F32, kind="ExternalOutput").ap()

    with tile.TileContext(nc) as tc:
        _emit(tc, x_d, w_d, ws_d, es_d, s_d)
    nc.compile()
    return nc


def _emit(tc, x_d, w_d, ws_d, es_d, s_d):
    nc = tc.nc
    from contextlib import ExitStack

    ctx = ExitStack()
    const = ctx.enter_context(tc.tile_pool(name="const", bufs=1))
    wpool = ctx.enter_context(tc.tile_pool(name="w", bufs=4))
    rpool = ctx.enter_context(tc.tile_pool(name="routing", bufs=4))
    spool = ctx.enter_context(tc.tile_pool(name="small", bufs=4))
    e2pool = ctx.enter_context(tc.tile_pool(name="e2", bufs=4))
    r2pool = ctx.enter_context(tc.tile_pool(name="r2", bufs=4))
    tmppool = ctx.enter_context(tc.tile_pool(name="tmp", bufs=12))
    ps1pool = ctx.enter_context(tc.tile_pool(name="ps1", bufs=1, space="PSUM"))
    psihpool = ctx.enter_context(tc.tile_pool(name="psih", bufs=6, space="PSUM"))
    psspool = ctx.enter_context(tc.tile_pool(name="pss", bufs=1, space="PSUM"))

    # resident inputs
    # x: block-diagonal stationary per (quad, half): [64 = (4n,16i), 128 = (4n,32b)]
    # loaded in per-block slices, interleaved with W so block k's inputs all
    # arrive ~2 block-periods before use
    x_sb = const.tile([64, NQ, 2, 128], BF16)
    ws_sb = const.tile([64, NQ, NO], BF16)
    es_sb = const.tile([128, 32], BF16)
    warm_sb = const.tile([64, 512], BF16)
    w_tiles = [None] * NBLOCKS

    def emit_dma_xws(blk, split=False):
        q0 = blk * WCHUNK_Q
        if split:
            # halve the first transfer so block 0's first pass-1 matmuls
            # (quads q0..q0+3) start as early as possible
            hq = WCHUNK_Q // 2
            nc.sync.dma_start(x_sb[:, q0:q0 + hq], x_d[:, q0:q0 + hq])
            nc.sync.dma_start(ws_sb[:, q0:q0 + hq], ws_d[:, q0:q0 + hq])
            nc.sync.dma_start(x_sb[:, q0 + hq:q0 + WCHUNK_Q],
                              x_d[:, q0 + hq:q0 + WCHUNK_Q])
            nc.sync.dma_start(ws_sb[:, q0 + hq:q0 + WCHUNK_Q],
                              ws_d[:, q0 + hq:q0 + WCHUNK_Q])
            return
        nc.sync.dma_start(ws_sb[:, q0:q0 + WCHUNK_Q], ws_d[:, q0:q0 + WCHUNK_Q])
        nc.sync.dma_start(x_sb[:, q0:q0 + WCHUNK_Q], x_d[:, q0:q0 + WCHUNK_Q])

    def emit_dma_w(blk, split=False):
        q0 = blk * WCHUNK_Q
        w_tile = wpool.tile([64, WCHUNK_Q, OD], BF16, tag="wt")
        if split:
            hq = WCHUNK_Q // 2
            nc.sync.dma_start(
                w_tile[:, 0:hq],
                w_d[q0:q0 + hq].rearrange("q p f -> p q f"))
            nc.sync.dma_start(
                w_tile[:, hq:WCHUNK_Q],
                w_d[q0 + hq:q0 + WCHUNK_Q].rearrange("q p f -> p q f"))
        else:
            nc.sync.dma_start(
                w_tile[:], w_d[q0:q0 + WCHUNK_Q].rearrange("q p f -> p q f")
            )
        w_tiles[blk] = w_tile

    def emit_dma(blk):
        emit_dma_xws(blk)
        emit_dma_w(blk)

    # s accumulator psum, lives across the whole pass 2.
    # [64 b, 512 = (d,o)]; batch-half h units accumulate partitions 32h:32h+32
    # (disjoint partition ranges -> subtile deps let the epilogue drain halves)
    ps_s = psspool.tile([64, OD], F32)
    s_written = [False, False]
    e_emitted = [0, 0]

    # PE warm-up/filler matmuls: keep the tensor engine busy (and its p-state
    # ramp hot) during pipeline fill. Only legal before the first real E.
    def warm_mm(cols):
        nc.tensor.matmul(
            ps_s[:, 0:cols],
            lhsT=warm_sb[:, 0:64],
            rhs=warm_sb[:, 0:cols],
            start=True, stop=True,
        )

    e2_blocks = [None] * NBLOCKS
    r2_blocks = [None] * NBLOCKS

    # deferred E-contractions: list of (u, tmp_tile)
    E_LAG = 11
    pending_e = []

    def flush_e(u_final, lag=None):
        lag = E_LAG if lag is None else lag
        while pending_e and (len(pending_e) > lag or u_final):
            eu, etmp = pending_e.pop(0)
            h = eu % 2
            e_emitted[h] += 1
            nc.tensor.matmul(
                ps_s[32 * h:32 * (h + 1), :],
                lhsT=es_sb[:],
                rhs=etmp.rearrange("p d o -> p (d o)"),
                start=not s_written[h], stop=(e_emitted[h] == NUNITS // 2),
            )
            s_written[h] = True

    # -------- software pipeline over blocks --------
    # Routing is staged in (blk, chunk) pieces so each in-order engine queue
    # matches data readiness. Block 0 uses 4-unit chunks to cut pipeline-fill
    # latency; steady blocks use one 16-unit chunk per stage:
    #   pass1(b):      ihsum matmuls (PE)
    #   e1_(b,c):      exp (ACT)
    #   z1r1(b,c):     reduce+recip (DVE)
    #   mid(b,c):      u1, t2 (Pool STT)
    #   e2_(b,c):      exp (ACT)
    #   z2r2(b,c):     reduce+recip (DVE)
    ps1_blocks = [None] * NBLOCKS
    e1_t, r1_t, e2_t, r2_t = {}, {}, {}, {}

    def chunks_of(blk):
        if blk == 0:
            return [(0, 4), (4, 4), (8, 8)]
        if blk == 1:
            return [(0, 8), (8, 8)]
        return [(0, 16)]

    def ckey(blk, j):
        if blk == 0:
            c0 = (j // 4) * 4 if j < 8 else 8
        elif blk == 1:
            c0 = (j // 8) * 8
        else:
            c0 = 0
        return (blk, c0), j - c0

    def pass1(blk):
        ps1 = ps1pool.tile([128, UNITS_PER_BLOCK * NO], F32)
        for j in range(UNITS_PER_BLOCK):
            u = blk * UNITS_PER_BLOCK + j
            q, h = u // 2, u % 2
            nc.tensor.matmul(
                ps1[:, 32 * j:32 * (j + 1)],
                lhsT=x_sb[:, q, h, :],
                rhs=ws_sb[:, q, :],
                start=True, stop=True,
            )
        ps1_blocks[blk] = ps1

    def ps1v(blk, c0, cs):
        return ps1_blocks[blk].rearrange(
            "p (j o) -> p j o", o=NO)[:, c0:c0 + cs, :]

    def e1_(blk, c0, cs):
        e1 = rpool.tile([128, cs, NO], BF16, tag=f"e1s{cs}c{c0}")
        nc.scalar.activation(e1[:], ps1v(blk, c0, cs), AF.Exp, scale=1.0 / 32.0)
        e1_t[(blk, c0)] = e1

    def z1r1(blk, c0, cs):
        e1 = e1_t[(blk, c0)]
        z1 = spool.tile([128, cs], F32, tag=f"z1s{cs}c{c0}")
        nc.vector.tensor_reduce(z1[:], e1[:], axis=mybir.AxisListType.X, op=OP.add)
        r1 = spool.tile([128, cs], F32, tag=f"r1s{cs}c{c0}")
        nc.vector.reciprocal(r1[:], z1[:])
        r1s = spool.tile([128, cs], F32, tag=f"r1x{cs}c{c0}")
        nc.vector.tensor_scalar_mul(r1s[:], r1[:], 32.0)
        r1_t[(blk, c0)] = r1s

    def mid(blk, c0, cs):
        # v32 = 32*c1 = e1 * (32*r1)  (gpsimd TT: the only legal Pool form)
        # t2raw = (v32 + 1) * ihsum   (DVE fused STT, reads ihsum from PSUM)
        # e2 = exp(t2raw / 32) = exp(ihsum*(1/32 + c1))
        e1 = e1_t[(blk, c0)]
        r1_b = r1_t[(blk, c0)][:, :, None].to_broadcast((128, cs, NO))
        v32 = rpool.tile([128, cs, NO], BF16, tag=f"u1s{cs}c{c0}")
        nc.gpsimd.tensor_tensor(v32[:], e1[:], r1_b, op=OP.mult)
        t2 = rpool.tile([128, cs, NO], BF16, tag=f"t2s{cs}c{c0}")
        nc.vector.scalar_tensor_tensor(t2[:], v32[:], 1.0, ps1v(blk, c0, cs),
                                       op0=OP.add, op1=OP.mult)
        e1_t[(blk, c0, "t2")] = t2

    def e2_(blk, c0, cs):
        t2 = e1_t[(blk, c0, "t2")]
        e2 = e2pool.tile([128, cs, NO], BF16, tag=f"e2s{cs}c{c0}")
        nc.scalar.activation(e2[:], t2[:], AF.Exp, scale=1.0 / 32.0)
        e2_t[(blk, c0)] = e2

    def z2r2(blk, c0, cs):
        e2 = e2_t[(blk, c0)]
        z2 = spool.tile([128, cs], F32, tag=f"z2s{cs}c{c0}")
        nc.vector.tensor_reduce(z2[:], e2[:], axis=mybir.AxisListType.X, op=OP.add)
        r2 = r2pool.tile([128, cs], F32, tag=f"r2s{cs}c{c0}")
        nc.vector.reciprocal(r2[:], z2[:])
        r2_t[(blk, c0)] = r2

    # weighting route per unit-slot: balance ACT/Pool/DVE under the PE floor
    #  A: ACT copy*r2 -> DVE TT*e2 (2x mode);  P: gpsimd STT;  B: DVE STT
    # route per unit-slot (gpsimd cannot read PSUM on real HW):
    #  B: DVE STT from PSUM;  A: ACT copy*r2 -> DVE TT*e2 (2x);
    #  C: ACT copy*r2 -> Pool STT*e2
    # B on even slots; A/C counts alternate by block parity so ACT/DVE/Pool
    # all amortize just under the PE floor.
    ROUTES_EVEN = ['B', 'A', 'C', 'B', 'A', 'C', 'B', 'A',
                    'C', 'B', 'A', 'C', 'A', 'B', 'C', 'B']
    ROUTES_ODD = ROUTES_EVEN
    # last block: finish with fast DVE evacs so the final E-train isn't gated
    # by a slow Pool multiply
    ROUTES_LAST = ['B', 'A', 'C', 'B', 'A', 'C', 'B', 'A',
                   'C', 'B', 'A', 'C', 'C', 'A', 'B', 'B']

    def emit_back(blk):
        """Pass-2 (ih matmuls, weighting, E-contraction) for one block,
        with the next block's routing stages interleaved at the right spots."""
        nxt = blk + 1 if blk + 1 < NBLOCKS else None
        if blk != 0:
            for c0, cs in chunks_of(blk):
                z2r2(blk, c0, cs)
            # pass1/e1 of the next block lead the PE/ACT queues this cycle,
            # so ps1 (single-buffered) is freed early and e1 is ready for z1.
            if nxt is not None:
                pass1(nxt)
                for c0, cs in chunks_of(nxt):
                    e1_(nxt, c0, cs)
        w_tile = w_tiles[blk]
        q0 = blk * WCHUNK_Q
        # interleave points for next-block routing stages (on their engines)
        if blk == 0:
            hooks = {1: "z2r2@4@4", 5: "z2r2@8@8",
                     6: "p1e1_nxt", 8: "z1r1", 12: "mid", 14: "e2"}
        else:
            hooks = {1: "z1r1", 3: "mid", 6: "e2"}
        if blk == NBLOCKS - 1:
            # drain batch-half h=1 first so the epilogue's h=1 copy/DMA
            # overlaps the remaining h=0 E-contractions
            j_order = [1, 3, 5, 7, 9, 11, 13, 15, 0, 2, 4, 6, 8, 10, 12, 14]
        else:
            j_order = list(range(UNITS_PER_BLOCK))
        for j in j_order:
            u = blk * UNITS_PER_BLOCK + j
            q, h = u // 2, u % 2
            ps_ih = psihpool.tile([128, OD], F32)
            nc.tensor.matmul(
                ps_ih[:],
                lhsT=x_sb[:, q, h, :],
                rhs=w_tile[:, q - q0, :],
                start=True, stop=True,
            )
            # psum free dim is (d, o); e2 broadcast over d has innermost step 1
            k, jj = ckey(blk, j)
            e2_b = e2_t[k][:, jj, None, :].to_broadcast((128, DO, NO))
            r2_s = r2_t[k][:, jj:jj + 1]
            tmp = tmppool.tile([128, DO, NO], BF16, tag="tmp")
            ps_v = ps_ih.rearrange("p (d o) -> p d o", o=NO)
            if blk == NBLOCKS - 1:
                rt = ROUTES_LAST[j]
            else:
                rt = (ROUTES_ODD if blk % 2 else ROUTES_EVEN)[j]
            if rt == 'B':
                nc.vector.scalar_tensor_tensor(
                    tmp[:], ps_v, r2_s, e2_b, op0=OP.mult, op1=OP.mult,
                )
            else:
                ihr = tmppool.tile([128, DO, NO], BF16, tag="ihr")
                nc.scalar.activation(ihr[:], ps_v, AF.Copy, scale=r2_s)
                if rt == 'C':
                    nc.gpsimd.tensor_tensor(tmp[:], ihr[:], e2_b, op=OP.mult)
                else:
                    nc.vector.tensor_tensor(tmp[:], ihr[:], e2_b, op=OP.mult)
            pending_e.append((u, tmp))
            flush_e(False)
            if WARM and blk == 0 and j <= 6:
                warm_mm(128)
                warm_mm(128)
                warm_mm(128)
            stage = hooks.get(j)
            if stage:
                if stage.startswith("z2r2@"):
                    _, zc0, zcs = stage.split("@")
                    z2r2(blk, int(zc0), int(zcs))
                elif nxt is None:
                    pass
                elif stage == "p1e1_nxt":
                    pass1(nxt)
                    for c0, cs in chunks_of(nxt):
                        e1_(nxt, c0, cs)
                else:
                    fn = {"z1r1": z1r1, "mid": mid, "e2": e2_}[stage]
                    for c0, cs in chunks_of(nxt):
                        fn(nxt, c0, cs)

    # prologue: block 0 dmas + chunked routing chain
    nc.gpsimd.memset(warm_sb[:], 0.0)
    if WARM:
        warm_mm(512)
        warm_mm(512)
        warm_mm(512)
    emit_dma_xws(0, split=True)
    emit_dma_w(0, split=True)
    emit_dma_xws(1)
    emit_dma_w(1)
    nc.sync.dma_start(es_sb[:], es_d[:])
    pass1(0)
    for c0, cs in chunks_of(0):
        e1_(0, c0, cs)
    for c0, cs in chunks_of(0):
        z1r1(0, c0, cs)
    for c0, cs in chunks_of(0):
        mid(0, c0, cs)
        e2_(0, c0, cs)
    z2r2(0, 0, 4)   # remaining blk-0 chunks are interleaved into back(0)
    emit_dma(2)
    for blk in range(NBLOCKS):
        if blk + 3 < NBLOCKS:
            emit_dma(blk + 3)
        emit_back(blk)
    flush_e(True)

    # ---------------- epilogue: s accumulated directly in ps_s [64, 512] ----
    # two partition-half copies/DMAs; the h=1 half only depends on h=1 Es
    # (subtile deps), which drain first in the reordered last block
    s_sb1 = const.tile([32, OD], F32)
    s_sb0 = const.tile([32, OD], F32)
    nc.scalar.copy(s_sb1[:, :], ps_s[32:64, :])
    nc.vector.tensor_copy(s_sb0[:, :], ps_s[0:32, :])
    nc.sync.dma_start(s_d[32:64, :], s_sb1[:, :])
    nc.sync.dma_start(s_d[0:32, :], s_sb0[:, :])
    ctx.close()


_NC_CACHE = None


def _get_program():
    global _NC_CACHE
    if _NC_CACHE is None:
        _NC_CACHE = _build_program()
    return _NC_CACHE


def kernel(inputs: np.ndarray, W: np.ndarray) -> np.ndarray:
    inputs = np.asarray(inputs, dtype=np.float32)
    W = np.asarray(W, dtype=np.float32)

    bf16 = ml_dtypes.bfloat16
    NQT = NI // 4  # quads over the full Ni
    # x block-diagonal stationaries: [NQT, 2, 4, 16, 4, 32] with blocks on the
    # (g, g) diagonal; block (q, h, g) = inputs[32h:32h+32, 4q+g, :].T
    xt = inputs.transpose(1, 2, 0)            # [Ni, Di, B]
    src = xt.reshape(NQT, 4, DI, 2, 32)       # [q, g, i, h, b]
    x4 = np.zeros((NQT, 2, 4, DI, 4, 32), dtype=np.float32)
    for g in range(4):
        x4[:, :, g, :, g, :] = src[:, g].transpose(0, 2, 1, 3)  # [q, h, i, b]
    x4 = x4.reshape(NQT, 2, 64, 128).transpose(2, 0, 1, 3)      # [64, q, h, 128]
    x4 = np.ascontiguousarray(x4).astype(bf16)
    # W: [1, Ni, No, Do, Di] -> [q, (g,i)=64, Do*No]  (columns are (d,o)-major)
    w4 = np.ascontiguousarray(
        W[0].transpose(0, 3, 2, 1).reshape(NQT, 4 * DI, OD)).astype(bf16)
    # Wsum over Do: [Ni, No, Di] -> [(g,i)=64, q, No]
    ws4 = W[0].sum(axis=2).transpose(0, 2, 1).reshape(NQT, 4 * DI, NO)
    ws4 = np.ascontiguousarray(ws4.transpose(1, 0, 2)).astype(bf16)  # [64, q, No]
    esel = np.tile(np.eye(32, dtype=np.float32), (4, 1)).astype(bf16)

    nc = _get_program()
    in_maps = []
    for c in range(NCORES):
        sl = slice(c * NQ, (c + 1) * NQ)
        in_maps.append({
            "x": np.ascontiguousarray(x4[:, sl]),
            "w": np.ascontiguousarray(w4[sl]),
            "ws": np.ascontiguousarray(ws4[:, sl]),
            "esel": esel,
        })
    res = run_bass_kernel_spmd(nc, in_maps, core_ids=list(range(NCORES)))
    s = np.zeros((64, OD), dtype=np.float32)
    for r in res.results:
        s += np.asarray(r["s_out"], dtype=np.float32)
    s = s.reshape(B, DO, NO).transpose(0, 2, 1)  # -> [B, No, Do]
    s2 = np.sum(np.square(s), axis=-1, keepdims=True)
    scale = s2 / (1.0 + s2) / np.sqrt(s2 + EPS)
    return (scale * s).astype(np.float32)



# revision 61
# speedup vs baseline: 1.0724x; 1.0724x over previous
"""CapsuleLayer (dynamic routing, 3 iterations) Trainium2 Bass kernel.

Problem: inputs [64, 2048, 16] f32, W [1, 2048, 32, 16, 16] f32
  inputs_hat[b,n,o,d] = sum_i W[n,o,d,i] * inputs[b,n,i]
  3 routing iterations (softmax over o); only the last s/squash matters, and the
  b-update never uses `outputs`, so the whole computation collapses to:
    ihsum[b,n,o] = sum_d ih[b,n,o,d]            (= x . Wsum)
    e1 = exp(ihsum/32); c1 = e1 / sum_o e1
    t2 = (c1 + 1/32)*ihsum  (= b2); e2 = exp(t2); r2 = 1/sum_o e2
    s[b,o,d] = sum_n e2*r2*ih
    out = squash(s)

Sharding: Ni (2048) split 8 ways (256 capsules per core). Routing is local per
(b, n); each core produces a partial s which the host sums and squashes.

Per-core device pipeline (matmuls bf16, PSUM f32), software-pipelined per
block of 16 units (unit = 4-capsule quad x 32-batch-half; x stationary is
block-diagonal [64=(4n,16i), 128=(4n,32b)]):
  pass 1: ihsum via per-unit matmuls into ps1 [128, 512] (1 bank).
  routing (chunked, interleaved into the previous block's pass 2 at hook
  positions; stages priority-boosted so they jump engine queues):
    e1 = ACT exp(ps1/32); z1,r1 DVE; v = c1 = e1*r1 (DVE TT);
    t2 = (v + 1/32)*ihsum (DVE STT, PSUM src); e2 = ACT exp(t2); z2,r2 DVE.
  pass 2, weighting routes per unit slot (tuned mix 7B/2A/7C):
    B: DVE STT ps*r2*e2 -> bf16 tmp (single op, PSUM src)
    A: ACT copy*r2 -> DVE TT*e2 (2x mode)
    C: ACT copy*r2 -> Pool TT*e2
  E-contraction (n-quad sum + b-transpose) on PE with tmp as the STATIONARY:
    4 chunk-matmuls per unit, lhsT = tmp[:, 128c:128c+128], rhs = es
    [128, 32] (0/1 selector), out free = 32 -> 4x cheaper than streaming tmp
    as the moving operand. Accumulates ps_sT[128, 2h, 4c, 32b'] (1 bank).
    NOTE: PSUM start/stop accumulation groups are per-BANK (a start=True on
    one region clobbers the bank's has_written) -> the whole bank is ONE
    group: start on the very first E matmul, stop on the 512th.
  last block: routes reordered (C's early, h=1 units first) so the h=1 drain
    copy/DMA overlaps h=0 compute and the final chain avoids slow Pool TTs.
  epilogue: ACT copies ps_sT halves to SBUF, DMA [128, 2, 4, 32] f32; host
    sums the 8 per-core partials, maps (m,h,c,b)->(b,o,d), applies squash.
"""

import os
import sys

import numpy as np
import ml_dtypes

sys.path.insert(0, "/opt/trn_rl_repo")
sys.path.insert(0, "/opt/pypackages")

import concourse.bass as bass
import concourse.mybir as mybir
import concourse.tile as tile
from concourse import bacc
from concourse.bass_utils import run_bass_kernel_spmd

BF16 = mybir.dt.bfloat16
F32 = mybir.dt.float32
AF = mybir.ActivationFunctionType
OP = mybir.AluOpType

B, NI, DI, NO, DO = 64, 2048, 16, 32, 16
NCORES = 8
NL = NI // NCORES            # 256 capsules per core
OD = NO * DO                 # 512
NQ = NL // 4                 # 64 quads (4 capsules each)
NUNITS = NQ * 2              # 128 units: (quad, batch-half)
UNITS_PER_BLOCK = 16         # routing block: 16 units -> psum [128, 512]
NBLOCKS = NUNITS // UNITS_PER_BLOCK   # 8
WCHUNK_Q = 8                 # quads per W dma chunk (32 capsules)
EPS = 1e-7

# weighting route per unit-slot (as baseline): B = DVE STT fused (PSUM src),
# A = ACT copy*r2 + DVE TT*e2 (2x), C = ACT copy*r2 + Pool TT*e2
ROUTES = list(os.environ.get('K_ROUTES', 'BCBABCCCBCBBACBC'))
# last block: Pool-heavy C routes go to the h=1 (early) phase so the final
# h=0 drain chain isn't gated by slow Pool TTs
ROUTES_LAST = list(os.environ.get('K_RLAST', 'BCABBCABBCAACBAA'))
E_LAG = int(os.environ.get('K_ELAG', '14'))
HOOKS_STEADY = os.environ.get('K_HOOKS', '1:z1r1,3:mid,6:e2')


def _build_program():
    nc = bacc.Bacc("TRN2", target_bir_lowering=False, debug=False)

    x_d = nc.dram_tensor("x", [64, NQ, 2, 128], BF16, kind="ExternalInput").ap()
    w_d = nc.dram_tensor("w", [NQ, 64, OD], BF16, kind="ExternalInput").ap()
    ws_d = nc.dram_tensor("ws", [64, NQ, NO], BF16, kind="ExternalInput").ap()
    es_d = nc.dram_tensor("esel", [128, 32], BF16, kind="ExternalInput").ap()
    s_d = nc.dram_tensor("s_out", [128, 2, 4, 32], F32,
                         kind="ExternalOutput").ap()
    dbg = None
    if os.environ.get('K_DEBUG'):
        dbg = {
            'ihs': nc.dram_tensor("d_ihs", [128, 16, 32], F32,
                                  kind="ExternalOutput").ap(),
            'e1': nc.dram_tensor("d_e1", [128, 16, 32], BF16,
                                 kind="ExternalOutput").ap(),
            't2': nc.dram_tensor("d_t2", [128, 16, 32], BF16,
                                 kind="ExternalOutput").ap(),
            'e2': nc.dram_tensor("d_e2", [128, 16, 32], BF16,
                                 kind="ExternalOutput").ap(),
            'r2': nc.dram_tensor("d_r2", [128, 16], F32,
                                 kind="ExternalOutput").ap(),
            'tmp0': nc.dram_tensor("d_tmp0", [128, 16, 32], BF16,
                                   kind="ExternalOutput").ap(),
        }

    with tile.TileContext(nc) as tc:
        _emit(tc, x_d, w_d, ws_d, es_d, s_d, dbg)
    nc.compile()
    return nc


def _emit(tc, x_d, w_d, ws_d, es_d, s_d, dbg=None):
    nc = tc.nc
    from contextlib import ExitStack

    ctx = ExitStack()
    const = ctx.enter_context(tc.tile_pool(name="const", bufs=1))
    wpool = ctx.enter_context(tc.tile_pool(name="w", bufs=4))
    rpool = ctx.enter_context(tc.tile_pool(name="routing", bufs=4))
    spool = ctx.enter_context(tc.tile_pool(name="small", bufs=4))
    e2pool = ctx.enter_context(tc.tile_pool(name="e2", bufs=4))
    r2pool = ctx.enter_context(tc.tile_pool(name="r2", bufs=4))
    tmppool = ctx.enter_context(tc.tile_pool(
        name="tmp", bufs=int(os.environ.get('K_TMPB', '12'))))
    ps1pool = ctx.enter_context(tc.tile_pool(
        name="ps1", bufs=int(os.environ.get('K_PS1B', '1')), space="PSUM"))
    psihpool = ctx.enter_context(tc.tile_pool(
        name="psih", bufs=int(os.environ.get('K_PSIHB', '6')), space="PSUM"))
    psspool = ctx.enter_context(tc.tile_pool(name="pss", bufs=1, space="PSUM"))

    # resident inputs
    # x: block-diagonal stationary per (quad, half): [64 = (4n,16i), 128 = (4n,32b)]
    x_sb = const.tile([64, NQ, 2, 128], BF16)
    ws_sb = const.tile([64, NQ, NO], BF16)
    es_sb = const.tile([128, 32], BF16)
    w_tiles = [None] * NBLOCKS

    def emit_dma_xws_range(qa, qb):
        nc.sync.dma_start(x_sb[:, qa:qb], x_d[:, qa:qb])
        nc.sync.dma_start(ws_sb[:, qa:qb], ws_d[:, qa:qb])

    def emit_dma_xws(blk):
        q0 = blk * WCHUNK_Q
        nc.sync.dma_start(ws_sb[:, q0:q0 + WCHUNK_Q], ws_d[:, q0:q0 + WCHUNK_Q])
        nc.sync.dma_start(x_sb[:, q0:q0 + WCHUNK_Q], x_d[:, q0:q0 + WCHUNK_Q])

    def emit_dma_w(blk, split=False):
        q0 = blk * WCHUNK_Q
        w_tile = wpool.tile([64, WCHUNK_Q, OD], BF16, tag="wt")
        if split:
            hq = WCHUNK_Q // 2
            nc.sync.dma_start(
                w_tile[:, 0:hq],
                w_d[q0:q0 + hq].rearrange("q p f -> p q f"))
            nc.sync.dma_start(
                w_tile[:, hq:WCHUNK_Q],
                w_d[q0 + hq:q0 + WCHUNK_Q].rearrange("q p f -> p q f"))
        else:
            nc.sync.dma_start(
                w_tile[:], w_d[q0:q0 + WCHUNK_Q].rearrange("q p f -> p q f")
            )
        w_tiles[blk] = w_tile

    def emit_dma(blk):
        emit_dma_xws(blk)
        emit_dma_w(blk)

    # s accumulator psum, transposed h-major layout: [128 = (4 dsub, 32 o),
    # 2 h, 4 dchunk, 32 b']; pass 2 accumulates into it via E chunk-matmuls.
    ps_sT = psspool.tile([128, 2, 4, 32], F32)
    s_sb1 = const.tile([128, 4, 32], F32)
    s_sb0 = const.tile([128, 4, 32], F32)
    e_started = {}
    e_count = {}

    # deferred E-contractions: list of (u, tmp_tile, rhs_ap)
    pending_e = []

    def flush_e(final, lag=None):
        # NOTE: start/stop accumulation groups in PSUM are per-BANK, not
        # per-element: a start=True on one (c,h) region clobbers the others'
        # accumulation state. So the whole bank is ONE group: start only on
        # the very first E matmul, stop only on the 512th (last); fresh
        # elements init via per-element has_written.
        lag = E_LAG if lag is None else lag
        while pending_e and (len(pending_e) > lag or final):
            eu, etmp, erhs = pending_e.pop(0)
            h = eu % 2
            tflat = etmp.rearrange("p d o -> p (d o)")
            for c in range(4):
                e_count['n'] = e_count.get('n', 0) + 1
                nc.tensor.matmul(
                    ps_sT[:, h, c, :],
                    lhsT=tflat[:, 128 * c:128 * (c + 1)],
                    rhs=erhs,
                    start=(e_count['n'] == 1),
                    stop=(e_count['n'] == NUNITS * 4),
                )

    # -------- routing stages (identical to baseline) --------
    ps1_blocks = [None] * NBLOCKS
    e1_t, r1_t, e2_t, r2_t = {}, {}, {}, {}

    B0C = os.environ.get('K_B0C', '4,4,8')
    B1C = os.environ.get('K_B1C', '8,8')

    def _chunklist(spec):
        out = []
        c0 = 0
        for cs in [int(x) for x in spec.split(',')]:
            out.append((c0, cs))
            c0 += cs
        return out

    def chunks_of(blk):
        if blk == 0:
            return _chunklist(B0C)
        if blk == 1:
            return _chunklist(B1C)
        return [(0, 16)]

    def ckey(blk, j):
        if blk == 0:
            c0 = (j // 4) * 4 if j < 8 else 8
        elif blk == 1:
            c0 = (j // 8) * 8
        else:
            c0 = 0
        return (blk, c0), j - c0

    PRIO = int(os.environ.get('K_PRIO', '40'))

    def prio():
        from contextlib import nullcontext
        return tc.high_priority(offset=PRIO) if PRIO > 0 else nullcontext()

    def pass1_chunk(blk, j0, cnt):
        ps1 = ps1_blocks[blk]
        for j in range(j0, j0 + cnt):
            u = blk * UNITS_PER_BLOCK + j
            q, h = u // 2, u % 2
            nc.tensor.matmul(
                ps1[:, 32 * j:32 * (j + 1)],
                lhsT=x_sb[:, q, h, :],
                rhs=ws_sb[:, q, :],
                start=True, stop=True,
            )

    def pass1(blk):
        ps1 = ps1pool.tile([128, UNITS_PER_BLOCK * NO], F32, tag="ps1")
        ps1_blocks[blk] = ps1
        pass1_chunk(blk, 0, UNITS_PER_BLOCK)

    def ps1v(blk, c0, cs):
        return ps1_blocks[blk].rearrange(
            "p (j o) -> p j o", o=NO)[:, c0:c0 + cs, :]

    def e1_(blk, c0, cs):
        e1 = rpool.tile([128, cs, NO], BF16, tag=f"e1s{cs}c{c0}")
        nc.scalar.activation(e1[:], ps1v(blk, c0, cs), AF.Exp, scale=1.0 / 32.0)
        e1_t[(blk, c0)] = e1

    def z1r1(blk, c0, cs):
        e1 = e1_t[(blk, c0)]
        z1 = spool.tile([128, cs], F32, tag=f"z1s{cs}c{c0}")
        nc.vector.tensor_reduce(z1[:], e1[:], axis=mybir.AxisListType.X, op=OP.add)
        r1 = spool.tile([128, cs], F32, tag=f"r1s{cs}c{c0}")
        nc.vector.reciprocal(r1[:], z1[:])
        r1_t[(blk, c0)] = r1

    def mid(blk, c0, cs):
        # v = c1 = e1 * r1           (TT with broadcast r1)
        # t2 = (v + 1/32) * ihsum    (DVE fused STT, reads ihsum from PSUM)
        # e2 = exp(t2) then needs scale 1.0 (the /32 is folded into 1/32 here)
        e1 = e1_t[(blk, c0)]
        r1_b = r1_t[(blk, c0)][:, :, None].to_broadcast((128, cs, NO))
        v32 = rpool.tile([128, cs, NO], BF16, tag=f"u1s{cs}c{c0}")
        if os.environ.get('K_V32', 'dve') == 'dve':
            nc.vector.tensor_tensor(v32[:], e1[:], r1_b, op=OP.mult)
        else:
            nc.gpsimd.tensor_tensor(v32[:], e1[:], r1_b, op=OP.mult)
        t2 = rpool.tile([128, cs, NO], BF16, tag=f"t2s{cs}c{c0}")
        nc.vector.scalar_tensor_tensor(t2[:], v32[:], 1.0 / 32.0,
                                       ps1v(blk, c0, cs),
                                       op0=OP.add, op1=OP.mult)
        e1_t[(blk, c0, "t2")] = t2

    def e2_(blk, c0, cs):
        t2 = e1_t[(blk, c0, "t2")]
        e2 = e2pool.tile([128, cs, NO], BF16, tag=f"e2s{cs}c{c0}")
        nc.scalar.activation(e2[:], t2[:], AF.Exp, scale=1.0)
        e2_t[(blk, c0)] = e2

    def z2r2(blk, c0, cs):
        e2 = e2_t[(blk, c0)]
        z2 = spool.tile([128, cs], F32, tag=f"z2s{cs}c{c0}")
        nc.vector.tensor_reduce(z2[:], e2[:], axis=mybir.AxisListType.X, op=OP.add)
        r2 = r2pool.tile([128, cs], F32, tag=f"r2s{cs}c{c0}")
        nc.vector.reciprocal(r2[:], z2[:])
        r2_t[(blk, c0)] = r2

    def emit_back(blk):
        """Pass-2 for one block: ih matmuls (singles + pairs), weighting
        routes, E chunk-matmuls; next block's routing stages interleaved."""
        nxt = blk + 1 if blk + 1 < NBLOCKS else None
        if blk != 0:
            for c0, cs in chunks_of(blk):
                if (blk, c0) not in r2_t:
                    z2r2(blk, c0, cs)
        if dbg is not None and blk == 2:
            nc.sync.dma_start(dbg['e1'], e1_t[(2, 0)][:])
            nc.sync.dma_start(dbg['t2'], e1_t[(2, 0, "t2")][:])
            nc.sync.dma_start(dbg['e2'], e2_t[(2, 0)][:])
            nc.sync.dma_start(dbg['r2'], r2_t[(2, 0)][:])
        if blk != 0 and nxt is not None:
            pass1(nxt)
            for c0, cs in chunks_of(nxt):
                e1_(nxt, c0, cs)
        w_tile = w_tiles[blk]
        q0 = blk * WCHUNK_Q
        if blk == 0:
            hooks = {1: "z2r2@4@4", 5: "z2r2@8@8",
                     6: "p1e1_nxt", 8: "z1r1", 12: "mid", 14: "e2"}
        else:
            hooks = {}
            for kv in HOOKS_STEADY.split(','):
                pos, stage = kv.split(':')
                hooks[int(pos)] = stage

        def unit_mm(j, out_ap):
            u = blk * UNITS_PER_BLOCK + j
            q = u // 2
            h = u % 2
            nc.tensor.matmul(
                out_ap,
                lhsT=x_sb[:, q, h, :],
                rhs=w_tile[:, q - q0, :],
                start=True, stop=True,
            )

        def e2r2_of(j):
            k, jj = ckey(blk, j)
            e2_b = e2_t[k][:, jj, None, :].to_broadcast((128, DO, NO))
            r2_s = r2_t[k][:, jj:jj + 1]
            return e2_b, r2_s

        def run_hook(j):
            stage = hooks.get(j)
            if not stage:
                return
            if stage.startswith("z2r2@"):
                _, zc0, zcs = stage.split("@")
                z2r2(blk, int(zc0), int(zcs))
            elif nxt is None:
                pass
            elif stage == "p1e1_nxt":
                pass1(nxt)
                for c0, cs in chunks_of(nxt):
                    e1_(nxt, c0, cs)
            else:
                fn = {"z1r1": z1r1, "mid": mid, "e2": e2_, "z2n": z2r2}[stage]
                for c0, cs in chunks_of(nxt):
                    fn(nxt, c0, cs)

        def emit_unit(j):
            u = blk * UNITS_PER_BLOCK + j
            ps_ih = psihpool.tile([128, OD], F32, tag="psih")
            unit_mm(j, ps_ih[:])
            e2_b, r2_s = e2r2_of(j)
            tmp = tmppool.tile([128, DO, NO], BF16, tag="tmp")
            ps_v = ps_ih.rearrange("p (d o) -> p d o", o=NO)
            rt = (ROUTES_LAST if blk == NBLOCKS - 1 else ROUTES)[j]
            if rt == 'B':
                nc.vector.scalar_tensor_tensor(
                    tmp[:], ps_v, r2_s, e2_b, op0=OP.mult, op1=OP.mult,
                )
            else:
                ihr = tmppool.tile([128, DO, NO], BF16, tag="ihr")
                nc.scalar.activation(ihr[:], ps_v, AF.Copy, scale=r2_s)
                if rt == 'C':
                    nc.gpsimd.tensor_tensor(tmp[:], ihr[:], e2_b, op=OP.mult)
                else:
                    nc.vector.tensor_tensor(tmp[:], ihr[:], e2_b, op=OP.mult)
            if dbg is not None and blk == 2 and j == 0:
                nc.sync.dma_start(dbg['tmp0'], tmp[:])
            pending_e.append((u, tmp, es_sb[:]))
            flush_e(False)

        if blk == NBLOCKS - 1:
            # h=1 units first, then h=1's epilogue copy/DMA overlaps the h=0
            # units; Es flushed eagerly so the final drain isn't E-gated
            for j in (1, 3, 5, 7, 9, 11, 13, 15):
                emit_unit(j)
            flush_e(True)
            nc.scalar.copy(s_sb1[:], ps_sT[:, 1])
            nc.sync.dma_start(s_d[:, 1], s_sb1[:])
            lblag = int(os.environ.get('K_LBLAG', '4'))
            for j in (0, 2, 4, 6, 8, 10, 12, 14):
                emit_unit(j)
                flush_e(False, lag=lblag)
        else:
            for j in range(UNITS_PER_BLOCK):
                emit_unit(j)
                run_hook(j)

    # prologue: block-0 dmas finely chunked so the routing chain starts as
    # early as possible; pass1 chunks interleaved with e1 chunks
    emit_dma_xws_range(0, 4)    # quads for units 0-7 (routing chunks (0,4)+(4,4))
    emit_dma_xws_range(4, 8)
    emit_dma_w(0, split=True)
    emit_dma_xws(1)
    emit_dma_w(1)
    nc.sync.dma_start(es_sb[:], es_d[:])
    # PE warm-up during the initial DMA latency: keeps pe_busy_start early so
    # block-0/1 matmuls run at full clock (ramp needs ~3us of PE busy).
    # Targets ps1's bank; pass1 overwrites the dead values afterwards.
    NWARM = int(os.environ.get('K_WARM', '0'))
    ps1 = ps1pool.tile([128, UNITS_PER_BLOCK * NO], F32, tag="ps1")
    ps1_blocks[0] = ps1
    if NWARM:
        warm_sb = const.tile([32, 32], BF16)
        nc.gpsimd.memset(warm_sb[:], 0.0)
        for i in range(NWARM):
            nc.tensor.matmul(ps1[0:32, 0:32], lhsT=warm_sb[:], rhs=warm_sb[:],
                             start=True, stop=True)
    for c0, cs in chunks_of(0):
        pass1_chunk(0, c0, cs)
        e1_(0, c0, cs)
    for c0, cs in chunks_of(0):
        z1r1(0, c0, cs)
    for c0, cs in chunks_of(0):
        mid(0, c0, cs)
        e2_(0, c0, cs)
    z2r2(0, 0, 4)   # remaining blk-0 chunks are interleaved into back(0)
    emit_dma(2)
    for blk in range(NBLOCKS):
        if blk + 3 < NBLOCKS:
            emit_dma(blk + 3)
        emit_back(blk)

    # ------ epilogue: A and B-h1 drains were emitted inside the last block --
    flush_e(True)
    nc.scalar.copy(s_sb0[:], ps_sT[:, 0])
    nc.sync.dma_start(s_d[:, 0], s_sb0[:])
    ctx.close()


_NC_CACHE = None


def _get_program():
    global _NC_CACHE
    if _NC_CACHE is None:
        _NC_CACHE = _build_program()
    return _NC_CACHE


def kernel(inputs: np.ndarray, W: np.ndarray) -> np.ndarray:
    inputs = np.asarray(inputs, dtype=np.float32)
    W = np.asarray(W, dtype=np.float32)

    bf16 = ml_dtypes.bfloat16
    NQT = NI // 4  # quads over the full Ni
    # x block-diagonal stationaries: [NQT, 2, 4, 16, 4, 32] with blocks on the
    # (g, g) diagonal; block (q, h, g) = inputs[32h:32h+32, 4q+g, :].T
    xt = inputs.transpose(1, 2, 0)            # [Ni, Di, B]
    src = xt.reshape(NQT, 4, DI, 2, 32)       # [q, g, i, h, b]
    x4 = np.zeros((NQT, 2, 4, DI, 4, 32), dtype=np.float32)
    for g in range(4):
        x4[:, :, g, :, g, :] = src[:, g].transpose(0, 2, 1, 3)  # [q, h, i, b]
    x4 = x4.reshape(NQT, 2, 64, 128).transpose(2, 0, 1, 3)      # [64, q, h, 128]
    x4 = np.ascontiguousarray(x4).astype(bf16)
    # W: [1, Ni, No, Do, Di] -> [q, (g,i)=64, Do*No]  (columns are (d,o)-major)
    w4 = np.ascontiguousarray(
        W[0].transpose(0, 3, 2, 1).reshape(NQT, 4 * DI, OD)).astype(bf16)
    # Wsum over Do: [Ni, No, Di] -> [(g,i)=64, q, No]
    ws4 = W[0].sum(axis=2).transpose(0, 2, 1).reshape(NQT, 4 * DI, NO)
    ws4 = np.ascontiguousarray(ws4.transpose(1, 0, 2)).astype(bf16)  # [64, q, No]
    esel = np.tile(np.eye(32, dtype=np.float32), (4, 1)).astype(bf16)

    nc = _get_program()
    in_maps = []
    for c in range(NCORES):
        sl = slice(c * NQ, (c + 1) * NQ)
        in_maps.append({
            "x": np.ascontiguousarray(x4[:, sl]),
            "w": np.ascontiguousarray(w4[sl]),
            "ws": np.ascontiguousarray(ws4[:, sl]),
            "esel": esel,
        })
    res = run_bass_kernel_spmd(nc, in_maps, core_ids=list(range(NCORES)))
    sT = np.zeros((128, 2, 4, 32), dtype=np.float32)
    for r in res.results:
        sT += np.asarray(r["s_out"], dtype=np.float32)
    # sT[m, h, c, b']: m = 32*dsub + o; b = 32h + b', d = 4*c + dsub
    s = sT.reshape(4, 32, 2, 4, 32).transpose(2, 4, 1, 3, 0).reshape(B, NO, DO)
    s2 = np.sum(np.square(s), axis=-1, keepdims=True)
    scale = s2 / (1.0 + s2) / np.sqrt(s2 + EPS)
    return (scale * s).astype(np.float32)


# revision 63
# speedup vs baseline: 1.0729x; 1.0004x over previous
"""CapsuleLayer (dynamic routing, 3 iterations) Trainium2 Bass kernel.

Problem: inputs [64, 2048, 16] f32, W [1, 2048, 32, 16, 16] f32
  inputs_hat[b,n,o,d] = sum_i W[n,o,d,i] * inputs[b,n,i]
  3 routing iterations (softmax over o); only the last s/squash matters, and the
  b-update never uses `outputs`, so the whole computation collapses to:
    ihsum[b,n,o] = sum_d ih[b,n,o,d]            (= x . Wsum)
    e1 = exp(ihsum/32); c1 = e1 / sum_o e1
    t2 = (c1 + 1/32)*ihsum  (= b2); e2 = exp(t2); r2 = 1/sum_o e2
    s[b,o,d] = sum_n e2*r2*ih
    out = squash(s)

Sharding: Ni (2048) split 8 ways (256 capsules per core). Routing is local per
(b, n); each core produces a partial s which the host sums and squashes.

Per-core device pipeline (matmuls bf16, PSUM f32), software-pipelined per
block of 16 units (unit = 4-capsule quad x 32-batch-half; x stationary is
block-diagonal [64=(4n,16i), 128=(4n,32b)]):
  pass 1: ihsum via per-unit matmuls into ps1 [128, 512] (1 bank).
  routing (chunked, interleaved into the previous block's pass 2 at hook
  positions; stages priority-boosted so they jump engine queues):
    e1 = ACT exp(ps1/32); z1,r1 DVE; v = c1 = e1*r1 (DVE TT);
    t2 = (v + 1/32)*ihsum (DVE STT, PSUM src); e2 = ACT exp(t2); z2,r2 DVE.
  pass 2, weighting routes per unit slot (tuned mix 7B/2A/7C):
    B: DVE STT ps*r2*e2 -> bf16 tmp (single op, PSUM src)
    A: ACT copy*r2 -> DVE TT*e2 (2x mode)
    C: ACT copy*r2 -> Pool TT*e2
  E-contraction (n-quad sum + b-transpose) on PE with tmp as the STATIONARY:
    4 chunk-matmuls per unit, lhsT = tmp[:, 128c:128c+128], rhs = es
    [128, 32] (0/1 selector), out free = 32 -> 4x cheaper than streaming tmp
    as the moving operand. Accumulates ps_sT[128, 2h, 4c, 32b'] (1 bank).
    NOTE: PSUM start/stop accumulation groups are per-BANK (a start=True on
    one region clobbers the bank's has_written) -> the whole bank is ONE
    group: start on the very first E matmul, stop on the 512th.
  last block: routes reordered (C's early, h=1 units first) so the h=1 drain
    copy/DMA overlaps h=0 compute and the final chain avoids slow Pool TTs.
  epilogue: ACT copies ps_sT halves to SBUF, DMA [128, 2, 4, 32] f32; host
    sums the 8 per-core partials, maps (m,h,c,b)->(b,o,d), applies squash.
"""

import os
import sys

import numpy as np
import ml_dtypes

sys.path.insert(0, "/opt/trn_rl_repo")
sys.path.insert(0, "/opt/pypackages")

import concourse.bass as bass
import concourse.mybir as mybir
import concourse.tile as tile
from concourse import bacc
from concourse.bass_utils import run_bass_kernel_spmd

BF16 = mybir.dt.bfloat16
F32 = mybir.dt.float32
AF = mybir.ActivationFunctionType
OP = mybir.AluOpType

B, NI, DI, NO, DO = 64, 2048, 16, 32, 16
NCORES = 8
NL = NI // NCORES            # 256 capsules per core
OD = NO * DO                 # 512
NQ = NL // 4                 # 64 quads (4 capsules each)
NUNITS = NQ * 2              # 128 units: (quad, batch-half)
UNITS_PER_BLOCK = 16         # routing block: 16 units -> psum [128, 512]
NBLOCKS = NUNITS // UNITS_PER_BLOCK   # 8
WCHUNK_Q = 8                 # quads per W dma chunk (32 capsules)
EPS = 1e-7

# weighting route per unit-slot (as baseline): B = DVE STT fused (PSUM src),
# A = ACT copy*r2 + DVE TT*e2 (2x), C = ACT copy*r2 + Pool TT*e2
ROUTES = list(os.environ.get('K_ROUTES', 'BCBABCCCBCBBACBC'))
# last block: Pool-heavy C routes go to the h=1 (early) phase so the final
# h=0 drain chain isn't gated by slow Pool TTs
ROUTES_LAST = list(os.environ.get('K_RLAST', 'BCABBCABBCAACBAA'))
E_LAG = int(os.environ.get('K_ELAG', '14'))
HOOKS_STEADY = os.environ.get('K_HOOKS', '1:z1r1,3:mid,6:e2')


def _build_program():
    nc = bacc.Bacc("TRN2", target_bir_lowering=False, debug=False)

    x_d = nc.dram_tensor("x", [64, NQ, 2, 128], BF16, kind="ExternalInput").ap()
    w_d = nc.dram_tensor("w", [NQ, 64, OD], BF16, kind="ExternalInput").ap()
    ws_d = nc.dram_tensor("ws", [64, NQ, NO], BF16, kind="ExternalInput").ap()
    es_d = nc.dram_tensor("esel", [128, 32], BF16, kind="ExternalInput").ap()
    s_d = nc.dram_tensor("s_out", [128, 2, 4, 32], F32,
                         kind="ExternalOutput").ap()
    dbg = None
    if os.environ.get('K_DEBUG'):
        dbg = {
            'ihs': nc.dram_tensor("d_ihs", [128, 16, 32], F32,
                                  kind="ExternalOutput").ap(),
            'e1': nc.dram_tensor("d_e1", [128, 16, 32], BF16,
                                 kind="ExternalOutput").ap(),
            't2': nc.dram_tensor("d_t2", [128, 16, 32], BF16,
                                 kind="ExternalOutput").ap(),
            'e2': nc.dram_tensor("d_e2", [128, 16, 32], BF16,
                                 kind="ExternalOutput").ap(),
            'r2': nc.dram_tensor("d_r2", [128, 16], F32,
                                 kind="ExternalOutput").ap(),
            'tmp0': nc.dram_tensor("d_tmp0", [128, 16, 32], BF16,
                                   kind="ExternalOutput").ap(),
        }

    with tile.TileContext(nc) as tc:
        _emit(tc, x_d, w_d, ws_d, es_d, s_d, dbg)
    nc.compile()
    return nc


def _emit(tc, x_d, w_d, ws_d, es_d, s_d, dbg=None):
    nc = tc.nc
    from contextlib import ExitStack

    ctx = ExitStack()
    const = ctx.enter_context(tc.tile_pool(name="const", bufs=1))
    wpool = ctx.enter_context(tc.tile_pool(name="w", bufs=4))
    rpool = ctx.enter_context(tc.tile_pool(name="routing", bufs=4))
    spool = ctx.enter_context(tc.tile_pool(name="small", bufs=4))
    e2pool = ctx.enter_context(tc.tile_pool(name="e2", bufs=4))
    r2pool = ctx.enter_context(tc.tile_pool(name="r2", bufs=4))
    tmppool = ctx.enter_context(tc.tile_pool(
        name="tmp", bufs=int(os.environ.get('K_TMPB', '12'))))
    ps1pool = ctx.enter_context(tc.tile_pool(
        name="ps1", bufs=int(os.environ.get('K_PS1B', '1')), space="PSUM"))
    psihpool = ctx.enter_context(tc.tile_pool(
        name="psih", bufs=int(os.environ.get('K_PSIHB', '6')), space="PSUM"))
    psspool = ctx.enter_context(tc.tile_pool(name="pss", bufs=1, space="PSUM"))

    # resident inputs
    # x: block-diagonal stationary per (quad, half): [64 = (4n,16i), 128 = (4n,32b)]
    x_sb = const.tile([64, NQ, 2, 128], BF16)
    ws_sb = const.tile([64, NQ, NO], BF16)
    es_sb = const.tile([128, 32], BF16)
    w_tiles = [None] * NBLOCKS

    def emit_dma_xws_range(qa, qb):
        nc.sync.dma_start(x_sb[:, qa:qb], x_d[:, qa:qb])
        nc.sync.dma_start(ws_sb[:, qa:qb], ws_d[:, qa:qb])

    def emit_dma_xws(blk):
        q0 = blk * WCHUNK_Q
        nc.sync.dma_start(ws_sb[:, q0:q0 + WCHUNK_Q], ws_d[:, q0:q0 + WCHUNK_Q])
        nc.sync.dma_start(x_sb[:, q0:q0 + WCHUNK_Q], x_d[:, q0:q0 + WCHUNK_Q])

    def emit_dma_w(blk, split=False):
        q0 = blk * WCHUNK_Q
        w_tile = wpool.tile([64, WCHUNK_Q, OD], BF16, tag="wt")
        if split:
            hq = WCHUNK_Q // 2
            nc.sync.dma_start(
                w_tile[:, 0:hq],
                w_d[q0:q0 + hq].rearrange("q p f -> p q f"))
            nc.sync.dma_start(
                w_tile[:, hq:WCHUNK_Q],
                w_d[q0 + hq:q0 + WCHUNK_Q].rearrange("q p f -> p q f"))
        else:
            nc.sync.dma_start(
                w_tile[:], w_d[q0:q0 + WCHUNK_Q].rearrange("q p f -> p q f")
            )
        w_tiles[blk] = w_tile

    def emit_dma(blk):
        emit_dma_xws(blk)
        emit_dma_w(blk)

    # s accumulator psum, transposed h-major layout: [128 = (4 dsub, 32 o),
    # 2 h, 4 dchunk, 32 b']; pass 2 accumulates into it via E chunk-matmuls.
    ps_sT = psspool.tile([128, 2, 4, 32], F32)
    s_sb1 = const.tile([128, 4, 32], F32)
    s_sb0 = const.tile([128, 4, 32], F32)
    e_started = {}
    e_count = {}

    # deferred E-contractions: list of (u, tmp_tile, rhs_ap)
    pending_e = []

    def flush_e(final, lag=None):
        # NOTE: start/stop accumulation groups in PSUM are per-BANK, not
        # per-element: a start=True on one (c,h) region clobbers the others'
        # accumulation state. So the whole bank is ONE group: start only on
        # the very first E matmul, stop only on the 512th (last); fresh
        # elements init via per-element has_written.
        lag = E_LAG if lag is None else lag
        while pending_e and (len(pending_e) > lag or final):
            eu, etmp, erhs = pending_e.pop(0)
            h = eu % 2
            tflat = etmp.rearrange("p d o -> p (d o)")
            for c in range(4):
                e_count['n'] = e_count.get('n', 0) + 1
                nc.tensor.matmul(
                    ps_sT[:, h, c, :],
                    lhsT=tflat[:, 128 * c:128 * (c + 1)],
                    rhs=erhs,
                    start=(e_count['n'] == 1),
                    stop=(e_count['n'] == NUNITS * 4),
                )

    # -------- routing stages (identical to baseline) --------
    ps1_blocks = [None] * NBLOCKS
    e1_t, r1_t, e2_t, r2_t = {}, {}, {}, {}

    B0C = os.environ.get('K_B0C', '2,6,8')
    B1C = os.environ.get('K_B1C', '8,8')

    def _chunklist(spec):
        out = []
        c0 = 0
        for cs in [int(x) for x in spec.split(',')]:
            out.append((c0, cs))
            c0 += cs
        return out

    def chunks_of(blk):
        if blk == 0:
            return _chunklist(B0C)
        if blk == 1:
            return _chunklist(B1C)
        return [(0, 16)]

    def ckey(blk, j):
        for c0, cs in chunks_of(blk):
            if c0 <= j < c0 + cs:
                return (blk, c0), j - c0
        raise AssertionError

    PRIO = int(os.environ.get('K_PRIO', '40'))

    def prio():
        from contextlib import nullcontext
        return tc.high_priority(offset=PRIO) if PRIO > 0 else nullcontext()

    def pass1_chunk(blk, j0, cnt):
        ps1 = ps1_blocks[blk]
        for j in range(j0, j0 + cnt):
            u = blk * UNITS_PER_BLOCK + j
            q, h = u // 2, u % 2
            nc.tensor.matmul(
                ps1[:, 32 * j:32 * (j + 1)],
                lhsT=x_sb[:, q, h, :],
                rhs=ws_sb[:, q, :],
                start=True, stop=True,
            )

    def pass1(blk):
        ps1 = ps1pool.tile([128, UNITS_PER_BLOCK * NO], F32, tag="ps1")
        ps1_blocks[blk] = ps1
        pass1_chunk(blk, 0, UNITS_PER_BLOCK)

    def ps1v(blk, c0, cs):
        return ps1_blocks[blk].rearrange(
            "p (j o) -> p j o", o=NO)[:, c0:c0 + cs, :]

    def e1_(blk, c0, cs):
        e1 = rpool.tile([128, cs, NO], BF16, tag=f"e1s{cs}c{c0}")
        nc.scalar.activation(e1[:], ps1v(blk, c0, cs), AF.Exp, scale=1.0 / 32.0)
        e1_t[(blk, c0)] = e1

    def z1r1(blk, c0, cs):
        e1 = e1_t[(blk, c0)]
        z1 = spool.tile([128, cs], F32, tag=f"z1s{cs}c{c0}")
        nc.vector.tensor_reduce(z1[:], e1[:], axis=mybir.AxisListType.X, op=OP.add)
        r1 = spool.tile([128, cs], F32, tag=f"r1s{cs}c{c0}")
        nc.vector.reciprocal(r1[:], z1[:])
        r1_t[(blk, c0)] = r1

    def mid(blk, c0, cs):
        # v = c1 = e1 * r1           (TT with broadcast r1)
        # t2 = (v + 1/32) * ihsum    (DVE fused STT, reads ihsum from PSUM)
        # e2 = exp(t2) then needs scale 1.0 (the /32 is folded into 1/32 here)
        e1 = e1_t[(blk, c0)]
        r1_b = r1_t[(blk, c0)][:, :, None].to_broadcast((128, cs, NO))
        v32 = rpool.tile([128, cs, NO], BF16, tag=f"u1s{cs}c{c0}")
        if os.environ.get('K_V32', 'dve') == 'dve':
            nc.vector.tensor_tensor(v32[:], e1[:], r1_b, op=OP.mult)
        else:
            nc.gpsimd.tensor_tensor(v32[:], e1[:], r1_b, op=OP.mult)
        t2 = rpool.tile([128, cs, NO], BF16, tag=f"t2s{cs}c{c0}")
        nc.vector.scalar_tensor_tensor(t2[:], v32[:], 1.0 / 32.0,
                                       ps1v(blk, c0, cs),
                                       op0=OP.add, op1=OP.mult)
        e1_t[(blk, c0, "t2")] = t2

    def e2_(blk, c0, cs):
        t2 = e1_t[(blk, c0, "t2")]
        e2 = e2pool.tile([128, cs, NO], BF16, tag=f"e2s{cs}c{c0}")
        nc.scalar.activation(e2[:], t2[:], AF.Exp, scale=1.0)
        e2_t[(blk, c0)] = e2

    def z2r2(blk, c0, cs):
        e2 = e2_t[(blk, c0)]
        z2 = spool.tile([128, cs], F32, tag=f"z2s{cs}c{c0}")
        nc.vector.tensor_reduce(z2[:], e2[:], axis=mybir.AxisListType.X, op=OP.add)
        r2 = r2pool.tile([128, cs], F32, tag=f"r2s{cs}c{c0}")
        nc.vector.reciprocal(r2[:], z2[:])
        r2_t[(blk, c0)] = r2

    def emit_back(blk):
        """Pass-2 for one block: ih matmuls (singles + pairs), weighting
        routes, E chunk-matmuls; next block's routing stages interleaved."""
        nxt = blk + 1 if blk + 1 < NBLOCKS else None
        if blk != 0:
            for c0, cs in chunks_of(blk):
                if (blk, c0) not in r2_t:
                    z2r2(blk, c0, cs)
        if dbg is not None and blk == 2:
            nc.sync.dma_start(dbg['e1'], e1_t[(2, 0)][:])
            nc.sync.dma_start(dbg['t2'], e1_t[(2, 0, "t2")][:])
            nc.sync.dma_start(dbg['e2'], e2_t[(2, 0)][:])
            nc.sync.dma_start(dbg['r2'], r2_t[(2, 0)][:])
        if blk != 0 and nxt is not None:
            pass1(nxt)
            for c0, cs in chunks_of(nxt):
                e1_(nxt, c0, cs)
        w_tile = w_tiles[blk]
        q0 = blk * WCHUNK_Q
        if blk == 0:
            hooks = {6: "p1e1_nxt", 8: "z1r1", 12: "mid", 14: "e2"}
            for c0, cs in chunks_of(0)[1:]:
                pos = max(0, c0 - 3)
                while pos in hooks:
                    pos += 1
                hooks[pos] = f"z2r2@{c0}@{cs}"
        else:
            hooks = {}
            for kv in HOOKS_STEADY.split(','):
                pos, stage = kv.split(':')
                hooks[int(pos)] = stage

        def unit_mm(j, out_ap):
            u = blk * UNITS_PER_BLOCK + j
            q = u // 2
            h = u % 2
            nc.tensor.matmul(
                out_ap,
                lhsT=x_sb[:, q, h, :],
                rhs=w_tile[:, q - q0, :],
                start=True, stop=True,
            )

        def e2r2_of(j):
            k, jj = ckey(blk, j)
            e2_b = e2_t[k][:, jj, None, :].to_broadcast((128, DO, NO))
            r2_s = r2_t[k][:, jj:jj + 1]
            return e2_b, r2_s

        def run_hook(j):
            stage = hooks.get(j)
            if not stage:
                return
            if stage.startswith("z2r2@"):
                _, zc0, zcs = stage.split("@")
                z2r2(blk, int(zc0), int(zcs))
            elif nxt is None:
                pass
            elif stage == "p1e1_nxt":
                pass1(nxt)
                for c0, cs in chunks_of(nxt):
                    e1_(nxt, c0, cs)
            else:
                fn = {"z1r1": z1r1, "mid": mid, "e2": e2_, "z2n": z2r2}[stage]
                for c0, cs in chunks_of(nxt):
                    fn(nxt, c0, cs)

        def emit_unit(j):
            u = blk * UNITS_PER_BLOCK + j
            ps_ih = psihpool.tile([128, OD], F32, tag="psih")
            unit_mm(j, ps_ih[:])
            e2_b, r2_s = e2r2_of(j)
            tmp = tmppool.tile([128, DO, NO], BF16, tag="tmp")
            ps_v = ps_ih.rearrange("p (d o) -> p d o", o=NO)
            rt = (ROUTES_LAST if blk == NBLOCKS - 1 else ROUTES)[j]
            if rt == 'B':
                nc.vector.scalar_tensor_tensor(
                    tmp[:], ps_v, r2_s, e2_b, op0=OP.mult, op1=OP.mult,
                )
            else:
                ihr = tmppool.tile([128, DO, NO], BF16, tag="ihr")
                nc.scalar.activation(ihr[:], ps_v, AF.Copy, scale=r2_s)
                if rt == 'C':
                    nc.gpsimd.tensor_tensor(tmp[:], ihr[:], e2_b, op=OP.mult)
                else:
                    nc.vector.tensor_tensor(tmp[:], ihr[:], e2_b, op=OP.mult)
            if dbg is not None and blk == 2 and j == 0:
                nc.sync.dma_start(dbg['tmp0'], tmp[:])
            pending_e.append((u, tmp, es_sb[:]))
            flush_e(False)

        if blk == NBLOCKS - 1:
            # h=1 units first, then h=1's epilogue copy/DMA overlaps the h=0
            # units; Es flushed eagerly so the final drain isn't E-gated
            for j in (1, 3, 5, 7, 9, 11, 13, 15):
                emit_unit(j)
            flush_e(True)
            nc.scalar.copy(s_sb1[:], ps_sT[:, 1])
            nc.sync.dma_start(s_d[:, 1], s_sb1[:])
            lblag = int(os.environ.get('K_LBLAG', '4'))
            for j in (0, 2, 4, 6, 8, 10, 12, 14):
                emit_unit(j)
                flush_e(False, lag=lblag)
        else:
            for j in range(UNITS_PER_BLOCK):
                emit_unit(j)
                run_hook(j)

    # prologue: block-0 dmas finely chunked so the routing chain starts as
    # early as possible; pass1 chunks interleaved with e1 chunks
    emit_dma_xws_range(0, 4)    # quads for units 0-7 (routing chunks (0,4)+(4,4))
    emit_dma_xws_range(4, 8)
    emit_dma_w(0, split=True)
    emit_dma_xws(1)
    emit_dma_w(1)
    nc.sync.dma_start(es_sb[:], es_d[:])
    # PE warm-up during the initial DMA latency: keeps pe_busy_start early so
    # block-0/1 matmuls run at full clock (ramp needs ~3us of PE busy).
    # Targets ps1's bank; pass1 overwrites the dead values afterwards.
    NWARM = int(os.environ.get('K_WARM', '0'))
    ps1 = ps1pool.tile([128, UNITS_PER_BLOCK * NO], F32, tag="ps1")
    ps1_blocks[0] = ps1
    if NWARM:
        warm_sb = const.tile([32, 32], BF16)
        nc.gpsimd.memset(warm_sb[:], 0.0)
        for i in range(NWARM):
            nc.tensor.matmul(ps1[0:32, 0:32], lhsT=warm_sb[:], rhs=warm_sb[:],
                             start=True, stop=True)
    for c0, cs in chunks_of(0):
        pass1_chunk(0, c0, cs)
        e1_(0, c0, cs)
    for c0, cs in chunks_of(0):
        z1r1(0, c0, cs)
    for c0, cs in chunks_of(0):
        mid(0, c0, cs)
        e2_(0, c0, cs)
    c00, cs0 = chunks_of(0)[0]
    z2r2(0, c00, cs0)   # remaining blk-0 chunks are interleaved into back(0)
    emit_dma(2)
    for blk in range(NBLOCKS):
        if blk + 3 < NBLOCKS:
            emit_dma(blk + 3)
        emit_back(blk)

    # ------ epilogue: A and B-h1 drains were emitted inside the last block --
    flush_e(True)
    nc.scalar.copy(s_sb0[:], ps_sT[:, 0])
    nc.sync.dma_start(s_d[:, 0], s_sb0[:])
    ctx.close()


_NC_CACHE = None


def _get_program():
    global _NC_CACHE
    if _NC_CACHE is None:
        _NC_CACHE = _build_program()
    return _NC_CACHE


def kernel(inputs: np.ndarray, W: np.ndarray) -> np.ndarray:
    inputs = np.asarray(inputs, dtype=np.float32)
    W = np.asarray(W, dtype=np.float32)

    bf16 = ml_dtypes.bfloat16
    NQT = NI // 4  # quads over the full Ni
    # x block-diagonal stationaries: [NQT, 2, 4, 16, 4, 32] with blocks on the
    # (g, g) diagonal; block (q, h, g) = inputs[32h:32h+32, 4q+g, :].T
    xt = inputs.transpose(1, 2, 0)            # [Ni, Di, B]
    src = xt.reshape(NQT, 4, DI, 2, 32)       # [q, g, i, h, b]
    x4 = np.zeros((NQT, 2, 4, DI, 4, 32), dtype=np.float32)
    for g in range(4):
        x4[:, :, g, :, g, :] = src[:, g].transpose(0, 2, 1, 3)  # [q, h, i, b]
    x4 = x4.reshape(NQT, 2, 64, 128).transpose(2, 0, 1, 3)      # [64, q, h, 128]
    x4 = np.ascontiguousarray(x4).astype(bf16)
    # W: [1, Ni, No, Do, Di] -> [q, (g,i)=64, Do*No]  (columns are (d,o)-major)
    w4 = np.ascontiguousarray(
        W[0].transpose(0, 3, 2, 1).reshape(NQT, 4 * DI, OD)).astype(bf16)
    # Wsum over Do: [Ni, No, Di] -> [(g,i)=64, q, No]
    ws4 = W[0].sum(axis=2).transpose(0, 2, 1).reshape(NQT, 4 * DI, NO)
    ws4 = np.ascontiguousarray(ws4.transpose(1, 0, 2)).astype(bf16)  # [64, q, No]
    esel = np.tile(np.eye(32, dtype=np.float32), (4, 1)).astype(bf16)

    nc = _get_program()
    in_maps = []
    for c in range(NCORES):
        sl = slice(c * NQ, (c + 1) * NQ)
        in_maps.append({
            "x": np.ascontiguousarray(x4[:, sl]),
            "w": np.ascontiguousarray(w4[sl]),
            "ws": np.ascontiguousarray(ws4[:, sl]),
            "esel": esel,
        })
    res = run_bass_kernel_spmd(nc, in_maps, core_ids=list(range(NCORES)))
    sT = np.zeros((128, 2, 4, 32), dtype=np.float32)
    for r in res.results:
        sT += np.asarray(r["s_out"], dtype=np.float32)
    # sT[m, h, c, b']: m = 32*dsub + o; b = 32h + b', d = 4*c + dsub
    s = sT.reshape(4, 32, 2, 4, 32).transpose(2, 4, 1, 3, 0).reshape(B, NO, DO)
    s2 = np.sum(np.square(s), axis=-1, keepdims=True)
    scale = s2 / (1.0 + s2) / np.sqrt(s2 + EPS)
    return (scale * s).astype(np.float32)
